# revision 26
# baseline (speedup 1.0000x reference)
"""Trainium2 Bass kernel for CorrelationModule (per-pixel self-attention).

Math (per batch element b, all fp32):
  xf = x[b] reshaped [C=384, N=2304]
  q = Wq@xf + bq, k = Wk@xf + bk, v = Wv@xf + bv       (1x1 convs)
  attn = softmax_m(q^T k / sqrt(512))                  (N x N)
  out = Wo @ (v @ attn^T) + bo                         -> [512, N]

Sharding: batch B=8 data-parallel across the 8 NeuronCores, params replicated.

Per-core kernel layout choices:
  - Wo is FOLDED into the V projection on the host: by associativity
    Wo@((Wv@xf)@attn^T) = ((Wo@Wv)@xf)@attn^T, so the kernel projects
    with Wv' = Wo@Wv and the AV matmul directly produces the final
    pre-normalize output -- the entire Wo projection (80 matmuls), the
    av->SBUF copies, and one PSUM bank disappear.  (bv folds into
    bo' = Wo@bv + bo since sum_m attn = 1; bk cancels in the softmax
    because its score contribution is constant along the reduction
    axis m; bq survives and is added to Q before quantization.)
  - Scores are computed TRANSPOSED: s_t[m, n] = sum_o k[o,m] q[o,n], so the
    softmax reduction (over m) lands on the PSUM partition axis.
  - The attention matmuls (scores and AV) run in fp8-e4m3 with
    perf_mode=DoubleRow: operands are stored as [128, 2, F] pair tiles
    (two 128-partition contraction groups per instruction), giving 2x
    the fp16 PE rate.  Numpy simulation of this exact quantization
    pipeline gives rel_err 1.56e-2 vs the 2e-2 gate (fp16 everywhere
    is 2.3e-4; quantizing the projection inputs too would be >2e-2, so
    the three projections stay fp16).
  - exp is taken without max-subtraction: scores*scale ~ N(0, 1/9), so
    exp() cannot overflow for this module's data distribution.
  - Deferred softmax normalization: the denominator is fp16-accumulated
    on DVE ([P,nw]), then partition-reduced AND broadcast in a single
    216ns PE matmul against an all-ones [128,128] stationary (vs a
    ~3.5us gpsimd partition_all_reduce), followed by a fast reciprocal.
    Emitted at each block's own tail, where the accumulator finishes
    anyway, so the normalize multiplies are ready long before the next
    block's AV matmuls need the av PSUM banks back.
  - The scores->AV dependency is software-pipelined by TWO m2 stages so
    the AV matmuls (which consume the whole e8 pair tile) never wait on
    the exp activations.
  - The previous block's bias-add+store chunks interleave at stages
    m2=3..6 of the current block, spacing them between exp activations
    in the ACT FIFO so exps are never queue-blocked.
  - A warmup burst of junk matmuls on a memset tile runs during the
    ~10us DMA preamble so the PE HAM clock-gate reaches 8/8 before the
    first real matmul (otherwise ~8us of K/V-proj run at 1.2GHz).
  - y is stored as fp16 (quantization ~6e-5 abs, negligible) and
    upcast to f32 on the host; halves the output DMA.  Stores alternate
    between the sync and (otherwise idle) gpsimd DMA queues.
"""

import numpy as np

B, C, O, H, W = 8, 384, 512, 48, 48
N = H * W  # 2304 tokens
P = 128
CT, OT, MT = C // P, O // P, N // P  # 3, 4, 18
M2 = MT // 2  # 9 DoubleRow m-pairs
NBLK = [(0, 512), (512, 512), (1024, 512), (1536, 512), (2048, 256)]
SCALE = 1.0 / float(np.sqrt(O))

_cache = {}


def _build_nc():
    import concourse.bacc as bacc
    import concourse.tile as tile
    import concourse.mybir as mybir

    F32 = mybir.dt.float32
    F16 = mybir.dt.float16
    F8 = mybir.dt.float8e4
    DR = mybir.MatmulPerfMode.DoubleRow

    nc = bacc.Bacc(
        "TRN2",
        target_bir_lowering=False,
        debug=False,
        enable_asserts=False,
        num_devices=1,
    )

    xf_d = nc.dram_tensor("xf", [C, N], F16, kind="ExternalInput").ap()
    wqkv_d = nc.dram_tensor("wqkv", [C, 3 * O], F16, kind="ExternalInput").ap()
    bias_d = nc.dram_tensor("bias", [O, 2], F32, kind="ExternalInput").ap()
    y_d = nc.dram_tensor("y", [O, N], F16, kind="ExternalOutput").ap()

    with tile.TileContext(nc) as tc:
        with (
            nc.allow_low_precision(reason="fp16/fp8 matmul operands"),
            tc.tile_pool(name="const", bufs=1) as const,
            tc.tile_pool(name="work", bufs=1) as work,
            tc.tile_pool(name="ps", bufs=1, space="PSUM") as ps,
        ):
            # ---- persistent SBUF tensors -------------------------------
            xf_sb = [
                const.tile([P, N], F16, tag=f"xf{c}", name=f"xf_sb{c}")
                for c in range(CT)
            ]
            wqkv_sb = [
                const.tile([P, 3 * O], F16, tag=f"wqkv{c}", name=f"wqkv_sb{c}")
                for c in range(CT)
            ]
            wqt_sb = [t[:, 0:O] for t in wqkv_sb]
            wkt_sb = [t[:, O:2 * O] for t in wqkv_sb]
            wvt_sb = [t[:, 2 * O:3 * O] for t in wqkv_sb]  # holds (Wo@Wv).T
            bias_sb = [
                const.tile([P, 2], F32, tag=f"bias{o}", name=f"bias_sb{o}")
                for o in range(OT)
            ]
            bq_sb = [t[:, 0:1] for t in bias_sb]
            bo2_sb = [t[:, 1:2] for t in bias_sb]
            # fp8 pair tiles: [p, i, f] holds contraction row i*128+p
            k8 = [
                const.tile([P, 2, N], F8, tag=f"k8_{o2}", name=f"k8_{o2}")
                for o2 in range(OT // 2)
            ]
            vt8 = [
                const.tile([P, 2, O], F8, tag=f"vt8_{m2}", name=f"vt8_{m2}")
                for m2 in range(M2)
            ]

            # ---- PE warmup: junk matmuls during the DMA preamble -------
            # (the first 128 cols double as the all-ones reduction operand)
            warm_sb = const.tile([P, 384], F16, tag="warm", name="warm_sb")
            nc.vector.memset(warm_sb[:], 1.0)
            ones_sb = warm_sb[:, 0:128]
            wps = ps.tile([P, 256], F32, tag="s", bufs=4, name="warm_ps")
            for i in range(16):
                nc.tensor.matmul(wps[:], warm_sb[:, 256:384],
                                 warm_sb[:, 0:256], start=True, stop=True)

            # ---- input loads -------------------------------------------
            # Tuned for time-to-first-matmul AND keeping the 8-14us HBM
            # window clear for the critical bytes (Wk + xf): scalar queue
            # carries only Wk; sync streams xf in progressive chunks
            # matched to the K-proj pace, then the cold weights.
            for c in range(CT):
                csl = slice(c * P, (c + 1) * P)
                nc.scalar.dma_start(wqkv_sb[c][:, O:2 * O],
                                    wqkv_d[csl, O:2 * O])
                nc.sync.dma_start(xf_sb[c][:, 0:512], xf_d[csl, 0:512])
            for o in range(OT):
                nc.gpsimd.dma_start(bias_sb[o][:], bias_d[o * P:(o + 1) * P, :])
            for c in range(CT):
                csl = slice(c * P, (c + 1) * P)
                nc.sync.dma_start(xf_sb[c][:, 512:1024], xf_d[csl, 512:1024])
            for c in range(CT):
                csl = slice(c * P, (c + 1) * P)
                nc.sync.dma_start(xf_sb[c][:, 1024:N], xf_d[csl, 1024:N])
            for c in range(CT):
                csl = slice(c * P, (c + 1) * P)
                nc.sync.dma_start(wqkv_sb[c][:, 2 * O:3 * O],
                                  wqkv_d[csl, 2 * O:3 * O])
            for c in range(CT):
                csl = slice(c * P, (c + 1) * P)
                nc.sync.dma_start(wqkv_sb[c][:, 0:O], wqkv_d[csl, 0:O])

            # ---- phase 1: K8 = fp8(Wk@xf), pair layout [o, m] ----------
            # (bk dropped: it cancels in the softmax over m)
            for n0, nw in NBLK:
                for o in range(OT):
                    osl = slice(o * P, (o + 1) * P)
                    kp = ps.tile([P, nw], F32, tag="s", bufs=4,
                                 name=f"kp_{o}_{n0}")
                    for c in range(CT):
                        nc.tensor.matmul(
                            kp[:],
                            wkt_sb[c][:, osl],
                            xf_sb[c][:, n0:n0 + nw],
                            start=(c == 0),
                            stop=(c == CT - 1),
                        )
                    dst = k8[o // 2][:, o % 2, n0:n0 + nw]
                    if o % 2 == 0:
                        nc.vector.tensor_copy(dst, kp[:])
                    else:
                        nc.scalar.copy(dst, kp[:])

            # ---- phase 1b: VT8 = fp8(((Wo@Wv)@xf)^T), layout [m, o] ----
            for m in range(MT):
                msl = slice(m * P, (m + 1) * P)
                vp = ps.tile([P, O], F32, tag="s", bufs=4, name=f"vp_{m}")
                for c in range(CT):
                    nc.tensor.matmul(
                        vp[:],
                        xf_sb[c][:, msl],
                        wvt_sb[c][:],
                        start=(c == 0),
                        stop=(c == CT - 1),
                    )
                dst = vt8[m // 2][:, m % 2, :]
                if m % 2 == 0:
                    nc.vector.tensor_copy(dst, vp[:])
                else:
                    nc.scalar.copy(dst, vp[:])

            # ---- phase 2: flash attention over n-blocks ----------------
            pending_norm = None  # denominator+muls of the prev block
            pending_outs = None  # bias-add+store chunks of the prev block
            for bi, (n0, nw) in enumerate(NBLK):
                nsl = slice(n0, n0 + nw)
                last = bi == len(NBLK) - 1
                # Q for this block: fp8 pair tiles [o, n], bias bq added
                q8 = [
                    work.tile([P, 2, nw], F8, tag=f"q8_{o2}", bufs=2,
                              name=f"q8_{n0}_{o2}")
                    for o2 in range(OT // 2)
                ]
                for o in range(OT):
                    osl = slice(o * P, (o + 1) * P)
                    qp = ps.tile([P, nw], F32, tag="s", bufs=4,
                                 name=f"qp_{n0}_{o}")
                    for c in range(CT):
                        nc.tensor.matmul(
                            qp[:],
                            wqt_sb[c][:, osl],
                            xf_sb[c][:, nsl],
                            start=(c == 0),
                            stop=(c == CT - 1),
                        )
                    nc.scalar.add(q8[o // 2][:, o % 2, :], qp[:], bq_sb[o])

                if pending_norm is not None:
                    pending_norm()
                    pending_norm = None

                av_ps = [
                    ps.tile([P, nw], F32, tag=f"av{o}", bufs=1,
                            name=f"av_{n0}_{o}")
                    for o in range(OT)
                ]
                # fp16 pair-width accumulator: one DVE add per m2 pair
                # (DVE ops here are overhead-dominated, so [P,2,nw] costs
                # barely more than [P,nw]); ~1e-3 relative rounding on the
                # ~2.3k-term positive sum.
                eacc2 = work.tile([P, 2, nw], F16, tag="eacc", bufs=2,
                                  name=f"eacc_{n0}")

                def emit_scores(m2, nw=nw, n0=n0, q8=q8):
                    e8 = work.tile([P, 2, nw], F8, tag="e8", bufs=4,
                                   name=f"e8_{n0}_{m2}")
                    for i in range(2):
                        m = 2 * m2 + i
                        msl = slice(m * P, (m + 1) * P)
                        sp = ps.tile([P, nw], F32, tag="s", bufs=4,
                                     name=f"sp_{n0}_{m}")
                        for o2 in range(OT // 2):
                            nc.tensor.matmul(
                                sp[:],
                                k8[o2][:, :, msl],
                                q8[o2][:],
                                start=(o2 == 0),
                                stop=(o2 == OT // 2 - 1),
                                perf_mode=DR,
                            )
                        nc.scalar.activation(
                            e8[:, i, :], sp[:],
                            mybir.ActivationFunctionType.Exp,
                            scale=SCALE,
                        )
                    return e8

                def emit_av(m2, e8, av_ps=av_ps):
                    for o in range(OT):
                        osl = slice(o * P, (o + 1) * P)
                        nc.tensor.matmul(
                            av_ps[o][:],
                            vt8[m2][:, :, osl],
                            e8[:],
                            start=(m2 == 0),
                            stop=(m2 == M2 - 1),
                            perf_mode=DR,
                        )

                # software-pipelined by TWO stages: scores(m2) before
                # av(m2-2), so the AV matmuls (which consume the whole e8
                # pair tile) never wait on the exp activations.  The
                # previous block's bias-add+store chunks land at m2=3..6.
                hist = []
                for m2 in range(M2):
                    e8 = emit_scores(m2)
                    if m2 >= 2:
                        emit_av(m2 - 2, hist[m2 - 2])
                    if pending_outs is not None and 3 <= m2 <= 6:
                        pending_outs(m2 - 3)
                    if m2 == 0:
                        nc.vector.tensor_copy(eacc2[:], e8[:])
                    else:
                        nc.vector.tensor_add(eacc2[:], eacc2[:], e8[:])
                    hist.append(e8)
                emit_av(M2 - 2, hist[M2 - 2])
                emit_av(M2 - 1, hist[M2 - 1])
                pending_outs = None

                # ---- denominator + normalize multiplies ----------------
                # partition-reduce AND broadcast in one PE matmul pair:
                # dsum[p,n] = sum_i sum_k ones[k,p]*eacc2[k,i,n], then a
                # fast reciprocal.  Deferred past the next block's Q-proj
                # (except for the last block) so the PE never waits on the
                # DVE eacc drain.
                tmps = []

                def make_norm(n0=n0, nw=nw, eacc2=eacc2, av_ps=av_ps,
                              tmps=tmps):
                    def norm():
                        dsum_ps = ps.tile([P, nw], F32, tag="s", bufs=4,
                                          name=f"dsum_{n0}")
                        for i in range(2):
                            nc.tensor.matmul(dsum_ps[:], ones_sb,
                                             eacc2[:, i, :],
                                             start=(i == 0), stop=(i == 1))
                        rb = work.tile([P, nw], F32, tag="rb_sb", bufs=2,
                                       name=f"rb_{n0}")
                        nc.vector.reciprocal_approx_fast(out=rb[:],
                                                         in_=dsum_ps[:])
                        for o in range(OT):
                            tmp = work.tile([P, nw], F32, tag="tmp", bufs=4,
                                            name=f"tmp_{n0}_{o}")
                            nc.vector.tensor_mul(tmp[:], av_ps[o][:], rb[:])
                            tmps.append(tmp)
                    return norm

                def make_outs(n0=n0, nw=nw, nsl=nsl, tmps=tmps,
                              sync_only=last):
                    def outs(p):
                        psl = slice(p * P, (p + 1) * P)
                        # 4 bufs: reusable only once the store DMA completes
                        outt = work.tile([P, nw], F16, tag="out", bufs=4,
                                         name=f"out_{n0}_{p}")
                        nc.scalar.add(outt[:], tmps[p][:], bo2_sb[p])
                        # last block: keep stores off gpsimd so its end-of-
                        # kernel drain isn't gated on a just-issued DMA
                        if p % 2 == 0 or sync_only:
                            nc.sync.dma_start(y_d[psl, nsl], outt[:])
                        else:
                            nc.gpsimd.dma_start(y_d[psl, nsl], outt[:])
                    return outs

                if last:
                    make_norm()()
                    outs = make_outs()
                    for p in range(OT):
                        outs(p)
                else:
                    pending_norm = make_norm()
                    pending_outs = make_outs()

    nc.compile()
    return nc


def get_nc():
    if "nc" not in _cache:
        _cache["nc"] = _build_nc()
    return _cache["nc"]


def make_in_maps(x, Wq, bq, Wk, bk, Wv, bv, Wo, bo):
    x = np.asarray(x, np.float32)
    Wq = np.asarray(Wq, np.float32)
    Wk = np.asarray(Wk, np.float32)
    Wv = np.asarray(Wv, np.float32)
    Wo = np.asarray(Wo, np.float32)
    bq = np.asarray(bq, np.float32)
    bv = np.asarray(bv, np.float32)
    bo = np.asarray(bo, np.float32)

    Wv2 = Wo @ Wv  # fold the output projection into the V projection
    wqkv = np.concatenate([Wq.T, Wk.T, Wv2.T], axis=1).astype(np.float16)
    bo2 = (Wo @ bv + bo).astype(np.float32)
    bias = np.stack([bq, bo2], axis=1).astype(np.float32)

    xf = x.reshape(B, C, N).astype(np.float16)
    shared = {
        "wqkv": np.ascontiguousarray(wqkv),
        "bias": np.ascontiguousarray(bias),
    }
    return [
        {"xf": np.ascontiguousarray(xf[b]), **shared} for b in range(B)
    ]


def kernel(x, Wq, bq, Wk, bk, Wv, bv, Wo, bo):
    from concourse import bass_utils

    nc = get_nc()
    in_maps = make_in_maps(x, Wq, bq, Wk, bk, Wv, bv, Wo, bo)
    res = bass_utils.run_bass_kernel_spmd(nc, in_maps, core_ids=list(range(B)))
    y = np.stack([res.results[b]["y"] for b in range(B)], axis=0)
    return np.ascontiguousarray(y.reshape(B, O, H, W).astype(np.float32))


# revision 27
# speedup vs baseline: 1.0129x; 1.0129x over previous
"""Trainium2 Bass kernel for CorrelationModule (per-pixel self-attention).

Math (per batch element b, all fp32):
  xf = x[b] reshaped [C=384, N=2304]
  q = Wq@xf + bq, k = Wk@xf + bk, v = Wv@xf + bv       (1x1 convs)
  attn = softmax_m(q^T k / sqrt(512))                  (N x N)
  out = Wo @ (v @ attn^T) + bo                         -> [512, N]

Sharding: batch B=8 data-parallel across the 8 NeuronCores, params replicated.

Per-core kernel layout choices:
  - Wo is FOLDED into the V projection on the host: by associativity
    Wo@((Wv@xf)@attn^T) = ((Wo@Wv)@xf)@attn^T, so the kernel projects
    with Wv' = Wo@Wv and the AV matmul directly produces the final
    pre-normalize output -- the entire Wo projection (80 matmuls), the
    av->SBUF copies, and one PSUM bank disappear.  (bv folds into
    bo' = Wo@bv + bo since sum_m attn = 1; bk cancels in the softmax
    because its score contribution is constant along the reduction
    axis m; bq survives and is added to Q before quantization.)
  - Scores are computed TRANSPOSED: s_t[m, n] = sum_o k[o,m] q[o,n], so the
    softmax reduction (over m) lands on the PSUM partition axis.
  - The attention matmuls (scores and AV) run in fp8-e4m3 with
    perf_mode=DoubleRow: operands are stored as [128, 2, F] pair tiles
    (two 128-partition contraction groups per instruction), giving 2x
    the fp16 PE rate.  Numpy simulation of this exact quantization
    pipeline gives rel_err 1.56e-2 vs the 2e-2 gate (fp16 everywhere
    is 2.3e-4; quantizing the projection inputs too would be >2e-2, so
    the three projections stay fp16).
  - exp is taken without max-subtraction: scores*scale ~ N(0, 1/9), so
    exp() cannot overflow for this module's data distribution.
  - Deferred softmax normalization: the denominator is fp16-accumulated
    on DVE ([P,nw]), then partition-reduced AND broadcast in a single
    216ns PE matmul against an all-ones [128,128] stationary (vs a
    ~3.5us gpsimd partition_all_reduce), followed by a fast reciprocal.
    Emitted at each block's own tail, where the accumulator finishes
    anyway, so the normalize multiplies are ready long before the next
    block's AV matmuls need the av PSUM banks back.
  - The scores->AV dependency is software-pipelined by TWO m2 stages so
    the AV matmuls (which consume the whole e8 pair tile) never wait on
    the exp activations.
  - The previous block's bias-add+store chunks interleave at stages
    m2=3..6 of the current block, spacing them between exp activations
    in the ACT FIFO so exps are never queue-blocked.
  - A warmup burst of junk matmuls on a memset tile runs during the
    ~10us DMA preamble so the PE HAM clock-gate reaches 8/8 before the
    first real matmul (otherwise ~8us of K/V-proj run at 1.2GHz).
  - y is stored as fp16 (quantization ~6e-5 abs, negligible) and
    upcast to f32 on the host; halves the output DMA.  Stores alternate
    between the sync and (otherwise idle) gpsimd DMA queues.
"""

import numpy as np

B, C, O, H, W = 8, 384, 512, 48, 48
N = H * W  # 2304 tokens
P = 128
CT, OT, MT = C // P, O // P, N // P  # 3, 4, 18
M2 = MT // 2  # 9 DoubleRow m-pairs
NBLK = [(0, 512), (512, 512), (1024, 512), (1536, 512), (2048, 256)]
SCALE = 1.0 / float(np.sqrt(O))

_cache = {}


def _build_nc():
    import concourse.bacc as bacc
    import concourse.tile as tile
    import concourse.mybir as mybir

    F32 = mybir.dt.float32
    F16 = mybir.dt.float16
    F8 = mybir.dt.float8e4
    DR = mybir.MatmulPerfMode.DoubleRow

    nc = bacc.Bacc(
        "TRN2",
        target_bir_lowering=False,
        debug=False,
        enable_asserts=False,
        num_devices=1,
    )

    xf_d = nc.dram_tensor("xf", [C, N], F16, kind="ExternalInput").ap()
    wqkv_d = nc.dram_tensor("wqkv", [C, 3 * O], F16, kind="ExternalInput").ap()
    bias_d = nc.dram_tensor("bias", [O, 2], F32, kind="ExternalInput").ap()
    y_d = nc.dram_tensor("y", [O, N], F16, kind="ExternalOutput").ap()

    with tile.TileContext(nc) as tc:
        with (
            nc.allow_low_precision(reason="fp16/fp8 matmul operands"),
            tc.tile_pool(name="const", bufs=1) as const,
            tc.tile_pool(name="work", bufs=1) as work,
            tc.tile_pool(name="ps", bufs=1, space="PSUM") as ps,
        ):
            # ---- persistent SBUF tensors -------------------------------
            xf_sb = [
                const.tile([P, N], F16, tag=f"xf{c}", name=f"xf_sb{c}")
                for c in range(CT)
            ]
            wqkv_sb = [
                const.tile([P, 3 * O], F16, tag=f"wqkv{c}", name=f"wqkv_sb{c}")
                for c in range(CT)
            ]
            wqt_sb = [t[:, 0:O] for t in wqkv_sb]
            wkt_sb = [t[:, O:2 * O] for t in wqkv_sb]
            wvt_sb = [t[:, 2 * O:3 * O] for t in wqkv_sb]  # holds (Wo@Wv).T
            bias_sb = [
                const.tile([P, 2], F32, tag=f"bias{o}", name=f"bias_sb{o}")
                for o in range(OT)
            ]
            bq_sb = [t[:, 0:1] for t in bias_sb]
            bo2_sb = [t[:, 1:2] for t in bias_sb]
            # fp8 pair tiles: [p, i, f] holds contraction row i*128+p
            k8 = [
                const.tile([P, 2, N], F8, tag=f"k8_{o2}", name=f"k8_{o2}")
                for o2 in range(OT // 2)
            ]
            vt8 = [
                const.tile([P, 2, O], F8, tag=f"vt8_{m2}", name=f"vt8_{m2}")
                for m2 in range(M2)
            ]

            # ---- PE warmup: junk matmuls during the DMA preamble -------
            # (the first 128 cols double as the all-ones reduction operand)
            warm_sb = const.tile([P, 384], F16, tag="warm", name="warm_sb")
            nc.vector.memset(warm_sb[:], 1.0)
            ones_sb = warm_sb[:, 0:128]
            wps = ps.tile([P, 256], F32, tag="s", bufs=4, name="warm_ps")
            for i in range(16):
                nc.tensor.matmul(wps[:], warm_sb[:, 256:384],
                                 warm_sb[:, 0:256], start=True, stop=True)

            # ---- input loads -------------------------------------------
            # Tuned for time-to-first-matmul AND keeping the 8-14us HBM
            # window clear for the critical bytes (Wk + xf): scalar queue
            # carries only Wk; sync streams xf in progressive chunks
            # matched to the K-proj pace, then the cold weights.
            for c in range(CT):
                csl = slice(c * P, (c + 1) * P)
                nc.scalar.dma_start(wqkv_sb[c][:, O:2 * O],
                                    wqkv_d[csl, O:2 * O])
                nc.sync.dma_start(xf_sb[c][:, 0:512], xf_d[csl, 0:512])
            for o in range(OT):
                nc.gpsimd.dma_start(bias_sb[o][:], bias_d[o * P:(o + 1) * P, :])
            for c in range(CT):
                csl = slice(c * P, (c + 1) * P)
                nc.sync.dma_start(xf_sb[c][:, 512:1024], xf_d[csl, 512:1024])
            for c in range(CT):
                csl = slice(c * P, (c + 1) * P)
                nc.sync.dma_start(xf_sb[c][:, 1024:N], xf_d[csl, 1024:N])
            for c in range(CT):
                csl = slice(c * P, (c + 1) * P)
                nc.sync.dma_start(wqkv_sb[c][:, 2 * O:3 * O],
                                  wqkv_d[csl, 2 * O:3 * O])
            for c in range(CT):
                csl = slice(c * P, (c + 1) * P)
                nc.sync.dma_start(wqkv_sb[c][:, 0:O], wqkv_d[csl, 0:O])

            # ---- phase 1: K8 = fp8(Wk@xf), pair layout [o, m] ----------
            # (bk dropped: it cancels in the softmax over m)
            for n0, nw in NBLK:
                for o in range(OT):
                    osl = slice(o * P, (o + 1) * P)
                    kp = ps.tile([P, nw], F32, tag="s", bufs=4,
                                 name=f"kp_{o}_{n0}")
                    for c in range(CT):
                        nc.tensor.matmul(
                            kp[:],
                            wkt_sb[c][:, osl],
                            xf_sb[c][:, n0:n0 + nw],
                            start=(c == 0),
                            stop=(c == CT - 1),
                        )
                    dst = k8[o // 2][:, o % 2, n0:n0 + nw]
                    if o % 2 == 0:
                        nc.vector.tensor_copy(dst, kp[:])
                    else:
                        nc.scalar.copy(dst, kp[:])

            # ---- phase 1b: VT8 = fp8(((Wo@Wv)@xf)^T), layout [m, o] ----
            for m in range(MT):
                msl = slice(m * P, (m + 1) * P)
                vp = ps.tile([P, O], F32, tag="s", bufs=4, name=f"vp_{m}")
                for c in range(CT):
                    nc.tensor.matmul(
                        vp[:],
                        xf_sb[c][:, msl],
                        wvt_sb[c][:],
                        start=(c == 0),
                        stop=(c == CT - 1),
                    )
                dst = vt8[m // 2][:, m % 2, :]
                if m % 2 == 0:
                    nc.vector.tensor_copy(dst, vp[:])
                else:
                    nc.scalar.copy(dst, vp[:])

            # ---- phase 2: flash attention over n-blocks ----------------
            pending_norm = None  # denominator+muls of the prev block
            pending_outs = None  # bias-add+store chunks of the prev block
            for bi, (n0, nw) in enumerate(NBLK):
                nsl = slice(n0, n0 + nw)
                last = bi == len(NBLK) - 1
                # Q for this block: fp8 pair tiles [o, n], bias bq added
                q8 = [
                    work.tile([P, 2, nw], F8, tag=f"q8_{o2}", bufs=2,
                              name=f"q8_{n0}_{o2}")
                    for o2 in range(OT // 2)
                ]
                for o in range(OT):
                    osl = slice(o * P, (o + 1) * P)
                    qp = ps.tile([P, nw], F32, tag="s", bufs=4,
                                 name=f"qp_{n0}_{o}")
                    for c in range(CT):
                        nc.tensor.matmul(
                            qp[:],
                            wqt_sb[c][:, osl],
                            xf_sb[c][:, nsl],
                            start=(c == 0),
                            stop=(c == CT - 1),
                        )
                    nc.scalar.add(q8[o // 2][:, o % 2, :], qp[:], bq_sb[o])

                if pending_norm is not None:
                    pending_norm()
                    pending_norm = None

                av_ps = [
                    ps.tile([P, nw], F32, tag=f"av{o}", bufs=1,
                            name=f"av_{n0}_{o}")
                    for o in range(OT)
                ]
                # fp16 pair-width accumulator: one DVE add per m2 pair
                # (DVE ops here are overhead-dominated, so [P,2,nw] costs
                # barely more than [P,nw]); ~1e-3 relative rounding on the
                # ~2.3k-term positive sum.
                eacc2 = work.tile([P, 2, nw], F16, tag="eacc", bufs=2,
                                  name=f"eacc_{n0}")

                def emit_scores(m2, nw=nw, n0=n0, q8=q8):
                    e8 = work.tile([P, 2, nw], F8, tag="e8", bufs=4,
                                   name=f"e8_{n0}_{m2}")
                    for i in range(2):
                        m = 2 * m2 + i
                        msl = slice(m * P, (m + 1) * P)
                        sp = ps.tile([P, nw], F32, tag="s", bufs=4,
                                     name=f"sp_{n0}_{m}")
                        for o2 in range(OT // 2):
                            nc.tensor.matmul(
                                sp[:],
                                k8[o2][:, :, msl],
                                q8[o2][:],
                                start=(o2 == 0),
                                stop=(o2 == OT // 2 - 1),
                                perf_mode=DR,
                            )
                        nc.scalar.activation(
                            e8[:, i, :], sp[:],
                            mybir.ActivationFunctionType.Exp,
                            scale=SCALE,
                        )
                    return e8

                def emit_av(m2, e8, av_ps=av_ps):
                    for o in range(OT):
                        osl = slice(o * P, (o + 1) * P)
                        nc.tensor.matmul(
                            av_ps[o][:],
                            vt8[m2][:, :, osl],
                            e8[:],
                            start=(m2 == 0),
                            stop=(m2 == M2 - 1),
                            perf_mode=DR,
                        )

                # software-pipelined by TWO stages: scores(m2) before
                # av(m2-2), so the AV matmuls (which consume the whole e8
                # pair tile) never wait on the exp activations.  The
                # previous block's bias-add+store chunks land at m2=3..6.
                hist = []
                for m2 in range(M2):
                    e8 = emit_scores(m2)
                    if m2 >= 2:
                        emit_av(m2 - 2, hist[m2 - 2])
                    if pending_outs is not None and 3 <= m2 <= 6:
                        pending_outs(m2 - 3)
                    if m2 == 0:
                        nc.vector.tensor_copy(eacc2[:], e8[:])
                    else:
                        nc.vector.tensor_add(eacc2[:], eacc2[:], e8[:])
                    hist.append(e8)
                emit_av(M2 - 2, hist[M2 - 2])
                emit_av(M2 - 1, hist[M2 - 1])
                pending_outs = None

                # ---- denominator + normalize multiplies ----------------
                # partition-reduce AND broadcast in one PE matmul pair:
                # dsum[p,n] = sum_i sum_k ones[k,p]*eacc2[k,i,n], then a
                # fast reciprocal.  Deferred past the next block's Q-proj
                # (except for the last block) so the PE never waits on the
                # DVE eacc drain.
                tmps = []

                def make_norm(n0=n0, nw=nw, eacc2=eacc2, av_ps=av_ps,
                              tmps=tmps):
                    def norm():
                        dsum_ps = ps.tile([P, nw], F32, tag="s", bufs=4,
                                          name=f"dsum_{n0}")
                        for i in range(2):
                            nc.tensor.matmul(dsum_ps[:], ones_sb,
                                             eacc2[:, i, :],
                                             start=(i == 0), stop=(i == 1))
                        rb = work.tile([P, nw], F32, tag="rb_sb", bufs=2,
                                       name=f"rb_{n0}")
                        nc.vector.reciprocal_approx_fast(out=rb[:],
                                                         in_=dsum_ps[:])
                        for o in range(OT):
                            tmp = work.tile([P, nw], F32, tag="tmp", bufs=4,
                                            name=f"tmp_{n0}_{o}")
                            nc.vector.tensor_mul(tmp[:], av_ps[o][:], rb[:])
                            tmps.append(tmp)
                    return norm

                def make_outs(n0=n0, nw=nw, nsl=nsl, tmps=tmps):
                    def outs(p):
                        psl = slice(p * P, (p + 1) * P)
                        # 4 bufs: reusable only once the store DMA completes
                        outt = work.tile([P, nw], F16, tag="out", bufs=4,
                                         name=f"out_{n0}_{p}")
                        nc.scalar.add(outt[:], tmps[p][:], bo2_sb[p])
                        if p % 2 == 0:
                            nc.sync.dma_start(y_d[psl, nsl], outt[:])
                        else:
                            nc.gpsimd.dma_start(y_d[psl, nsl], outt[:])
                    return outs

                if last:
                    make_norm()()
                    outs = make_outs()
                    for p in range(OT):
                        outs(p)
                else:
                    pending_norm = make_norm()
                    pending_outs = make_outs()

    nc.compile()
    return nc


def get_nc():
    if "nc" not in _cache:
        _cache["nc"] = _build_nc()
    return _cache["nc"]


def make_in_maps(x, Wq, bq, Wk, bk, Wv, bv, Wo, bo):
    x = np.asarray(x, np.float32)
    Wq = np.asarray(Wq, np.float32)
    Wk = np.asarray(Wk, np.float32)
    Wv = np.asarray(Wv, np.float32)
    Wo = np.asarray(Wo, np.float32)
    bq = np.asarray(bq, np.float32)
    bv = np.asarray(bv, np.float32)
    bo = np.asarray(bo, np.float32)

    Wv2 = Wo @ Wv  # fold the output projection into the V projection
    wqkv = np.concatenate([Wq.T, Wk.T, Wv2.T], axis=1).astype(np.float16)
    bo2 = (Wo @ bv + bo).astype(np.float32)
    bias = np.stack([bq, bo2], axis=1).astype(np.float32)

    xf = x.reshape(B, C, N).astype(np.float16)
    shared = {
        "wqkv": np.ascontiguousarray(wqkv),
        "bias": np.ascontiguousarray(bias),
    }
    return [
        {"xf": np.ascontiguousarray(xf[b]), **shared} for b in range(B)
    ]


def kernel(x, Wq, bq, Wk, bk, Wv, bv, Wo, bo):
    from concourse import bass_utils

    nc = get_nc()
    in_maps = make_in_maps(x, Wq, bq, Wk, bk, Wv, bv, Wo, bo)
    res = bass_utils.run_bass_kernel_spmd(nc, in_maps, core_ids=list(range(B)))
    y = np.stack([res.results[b]["y"] for b in range(B)], axis=0)
    return np.ascontiguousarray(y.reshape(B, O, H, W).astype(np.float32))
